# revision 59
# baseline (speedup 1.0000x reference)
"""TRN2 Bass kernel for nn_DiffQuantumSimulator (QAOA MaxCut, 18 qubits, p=4).

Strategy: data-parallel over batch (8 graphs -> 8 NeuronCores). Per core the
2^18 statevector lives in SBUF as one [128 x 4096] fp16 tile (re | im halves).

Each QAOA layer applies exp(-i*hp) (diagonal, elementwise) and the mixer
RX(beta)^(x)18 in 3 TensorE matmul phases:
  A: 128x128 complex gate RX^(x)7 on the 7 partition bits, fused with a
     partition<->free-bit transpose by using the *state* as the stationary
     operand (out = state_tile^T @ [C|D]).
  B: same trick on the next 7 bits (stride-16 windows).
  C: standard matmul applying RX^(x)4 (x) I_8 to the remaining 4 bits.
All matmuls run in fp16 (1 col/cycle at 2.4 GHz warm). The PE is pre-warmed
with dummy matmuls during the input-DMA head, and small fillers at the two
per-layer stall points hold the HAM clock-gate at K=8/8.

The diagonal rotation (layers 2..4) reads phase C's PSUM output: ScalarE
stages PSUM->SBUF fp16 (one [P,1024] copy per chunk), then VectorE applies
exp(-i hp) with one wide + two half multiplies and two add/subs, all in
16-bit 2x mode. C chunks and their rotations are emitted interleaved with
the B groups so the chain overlaps the whole B+C matmul stretch. PSUM->SBUF
copies for phases A/B are split across ScalarE/VectorE; input DMAs are
issued only from sync (HWDGE) and gpsimd (SWDGE) so the compute queues
never block on transfers.

Diagonals (cos/sin of hp per layer, in the layer's bit-layout), the gate
matrices, and hp itself are precomputed on host from the runtime inputs.
Device returns per-partition energy partial sums; host reduces and scales.
"""

import numpy as np

import concourse.bass as bass
import concourse.mybir as mybir
import concourse.tile as tile
from concourse import bacc
from concourse.bass_utils import run_bass_kernel_spmd

N = 18
DIM = 1 << N
P = 128
F = DIM // P  # 2048
LAYERS = 4
BATCH = 8
NCORES = 8

FP32 = mybir.dt.float32
FP16 = mybir.dt.float16
ALU = mybir.AluOpType
ACT = mybir.ActivationFunctionType

# ----------------------------------------------------------------------------
# Host-side math: hp diagonal, gate matrices, bit-layout permutations
# ----------------------------------------------------------------------------


def _compute_hp(adj):
    W = (np.triu(adj, k=1) > 0.5).astype(np.float64)
    n_edges = W.sum()
    idx = np.arange(DIM)
    shifts = (N - 1 - np.arange(N))[:, None]
    Z = 1.0 - 2.0 * ((idx[None, :] >> shifts) & 1).astype(np.float64)
    T = W @ Z
    cross = np.einsum("ud,ud->d", T, Z)
    return 0.5 * (n_edges - cross)  # [DIM], integer-valued*0.5, exact


def _rx(beta):
    c, s = np.cos(beta), np.sin(beta)
    return np.array([[c, -1j * s], [-1j * s, c]], dtype=np.complex128)


def _kron_list(mats):
    out = np.array([[1.0]], dtype=np.complex128)
    for m in mats:
        out = np.kron(out, m)
    return out


def _m7(beta):
    return _kron_list([_rx(beta)] * 7)


def _m41(beta):
    return _kron_list([_rx(beta)] * 4 + [np.eye(2, dtype=np.complex128)] * 3)


def _bitmap_after_A(bm):
    new = [0] * N
    for j in range(7):
        new[11 + j] = bm[j]
    for j in range(4):
        new[7 + j] = bm[7 + j]
    for j in range(7):
        new[j] = bm[11 + j]
    return new


def _bitmap_after_B(bm):
    # window = free bits 10..4 (stride-16 single AP dim), tiles = bits 3..0
    new = [0] * N
    for j in range(7):
        new[11 + j] = bm[4 + j]
    for j in range(4):
        new[7 + j] = bm[j]
    for j in range(7):
        new[j] = bm[11 + j]
    return new


def _perm_for_bitmap(bm):
    a = np.arange(DIM, dtype=np.int64)
    out = np.zeros(DIM, dtype=np.int64)
    for j in range(N):
        out |= ((a >> j) & 1) << bm[j]
    return out


def _layer_perms():
    """Permutations (orig_idx = perm[cur_idx]) for the state layout at the
    start of each layer (1..LAYERS) plus the final layout (index LAYERS)."""
    perms = []
    bm = list(range(N))
    for _ in range(LAYERS):
        perms.append(_perm_for_bitmap(bm))
        bm = _bitmap_after_B(_bitmap_after_A(bm))
    perms.append(_perm_for_bitmap(bm))
    return perms


_PERMS = _layer_perms()


def _host_prep(batch_betas, adj_matrices):
    """Build per-core input dicts."""
    in_maps = []
    for b in range(BATCH):
        hp = _compute_hp(np.asarray(adj_matrices[b], dtype=np.float64))
        cos_hp = np.cos(hp)
        sin_hp = np.sin(hp)

        # init state (= exp(-i hp) applied to unnormalized uniform state),
        # packed [P, 4096] = [re(2048) | im(2048)], matching the st_a layout
        init = np.empty((P, 4096), dtype=np.float16)
        init[:, 0:2048] = cos_hp[_PERMS[0]].reshape(P, F)
        init[:, 2048:4096] = (-sin_hp[_PERMS[0]]).reshape(P, F)

        # diag planes per rotation layer: dA = [cos|sin] per 512-chunk
        # (matches the C-output [re|im] chunk layout for one-op multiplies)
        diags = np.empty((LAYERS - 1, P, 4 * 1024), dtype=np.float16)
        for t in range(1, LAYERS):
            c = cos_hp[_PERMS[t]].reshape(P, 4, 512)
            s = sin_hp[_PERMS[t]].reshape(P, 4, 512)
            dA = diags[t - 1].reshape(P, 4, 2, 512)
            dA[:, :, 0, :] = c
            dA[:, :, 1, :] = s
        hp_plane = hp[_PERMS[LAYERS]].reshape(P, F).astype(np.float16)

        gates_ab = np.empty((P, LAYERS * 512), dtype=np.float16)
        gates_c = np.empty((P, LAYERS * 384), dtype=np.float16)
        for t in range(LAYERS):
            beta = float(np.asarray(batch_betas[b][t], dtype=np.float64))
            M7 = _m7(beta)
            C7 = M7.real
            D7 = M7.imag
            M41 = _m41(beta)
            C41 = M41.real
            D41 = M41.imag
            o = 512 * t
            gates_ab[:, o + 0 : o + 128] = C7
            gates_ab[:, o + 128 : o + 256] = D7
            gates_ab[:, o + 256 : o + 384] = -D7
            gates_ab[:, o + 384 : o + 512] = C7
            o = 384 * t
            gates_c[:, o + 0 : o + 128] = C41
            gates_c[:, o + 128 : o + 256] = -D41
            gates_c[:, o + 256 : o + 384] = D41

        in_maps.append(
            {
                "init": init,
                "diags": diags,
                "hp": hp_plane,
                "gates_ab": gates_ab,
                "gates_c": gates_c,
            }
        )
    return in_maps


# ----------------------------------------------------------------------------
# Bass program
# ----------------------------------------------------------------------------

N_WARMUP_MM = 6  # dummy matmuls (N=512) to warm HAM during the DMA head
ROT_ADD_GPSIMD = False  # GpSimd TT contends with DVE SBUF ports: net loss


def _build_program():
    nc = bacc.Bacc("TRN2", target_bir_lowering=False, debug=False)

    d_init = nc.dram_tensor("init", [P, 4096], FP16, kind="ExternalInput")
    d_diags = nc.dram_tensor(
        "diags", [LAYERS - 1, P, 4096], FP16, kind="ExternalInput"
    )
    d_hp = nc.dram_tensor("hp", [P, F], FP16, kind="ExternalInput")
    d_gab = nc.dram_tensor("gates_ab", [P, LAYERS * 512], FP16, kind="ExternalInput")
    d_gc = nc.dram_tensor("gates_c", [P, LAYERS * 384], FP16, kind="ExternalInput")
    d_out = nc.dram_tensor("out", [P, 1], FP32, kind="ExternalOutput")

    with tile.TileContext(nc) as tc:
        with (
            tc.tile_pool(name="state", bufs=1) as st_pool,
            tc.tile_pool(name="consts", bufs=1) as c_pool,
            tc.tile_pool(name="rot", bufs=2) as r_pool,
            tc.tile_pool(name="scratch", bufs=1) as s_pool,
            tc.tile_pool(name="ps_mm", bufs=3, space="PSUM") as ps_mm,
            tc.tile_pool(name="ps_c", bufs=2, space="PSUM") as ps_c,
            tc.tile_pool(name="ps_d", bufs=1, space="PSUM") as ps_d,
        ):
            # state tiles: [re(2048) | im(2048)]
            st_a = st_pool.tile([P, 2 * F], FP16, tag="st_a")
            st_b = st_pool.tile([P, 2 * F], FP16, tag="st_b")

            diag_t = [
                c_pool.tile([P, 4096], FP16, tag=f"diag{t}", name=f"diag{t}")
                for t in range(LAYERS - 1)
            ]
            hp_t = c_pool.tile([P, F], FP16, tag="hp")
            gab_all = c_pool.tile([P, LAYERS * 512], FP16, tag="gab")
            gc_all = c_pool.tile([P, LAYERS * 384], FP16, tag="gc")

            # rotation scratch (per chunk, double-buffered via pool bufs)
            wu = s_pool.tile([P, 512], FP16, tag="wu")
            sq_re = s_pool.tile([P, 512], FP16, tag="sq_re")
            sq_im = s_pool.tile([P, 512], FP16, tag="sq_im")
            probs = s_pool.tile([P, 512], FP16, tag="probs")
            part_k = [
                s_pool.tile([P, 1], FP32, tag=f"part{k}", name=f"part{k}")
                for k in range(4)
            ]
            comb = [
                s_pool.tile([P, 1], FP32, tag=f"comb{k}", name=f"comb{k}")
                for k in range(3)
            ]

            # ---- PE warmup / HAM-hold fillers: dummy matmuls on zeroed SBUF
            # into a dedicated PSUM bank (no deps on real work)
            nc.vector.memset(wu[:], 0.0)
            dps = ps_d.tile([P, 512], FP32, tag="ps_d")

            def dummy_mms(n, cols=256):
                for _ in range(n):
                    nc.tensor.matmul(
                        dps[:, 0:cols], wu[:, 0:128], wu[:, 0:cols],
                        start=True, stop=True,
                    )

            dummy_mms(N_WARMUP_MM, cols=512)

            # ---- input DMAs. NONE on scalar/vector (their queues must stay
            # free for compute). sync = HWDGE, gpsimd = SWDGE. Chunked so the
            # first pieces arrive early and phase A can start streaming.
            nc.sync.dma_start(gab_all[:, 0:512], d_gab.ap()[:, 0:512])
            nc.gpsimd.dma_start(st_a[:, 2048:2560], d_init.ap()[:, 2048:2560])
            for k in range(4):
                nc.sync.dma_start(
                    st_a[:, 512 * k : 512 * (k + 1)],
                    d_init.ap()[:, 512 * k : 512 * (k + 1)],
                )
                if k > 0:
                    nc.gpsimd.dma_start(
                        st_a[:, 2048 + 512 * k : 2048 + 512 * (k + 1)],
                        d_init.ap()[:, 2048 + 512 * k : 2048 + 512 * (k + 1)],
                    )
            nc.sync.dma_start(gc_all[:, 0:384], d_gc.ap()[:, 0:384])
            nc.sync.dma_start(gab_all[:, 512:1024], d_gab.ap()[:, 512:1024])
            # layer-2 rotation diag, chunked for earliest first-chunk arrival
            for h in range(4):
                sl = slice(1024 * h, 1024 * (h + 1))
                eng = nc.sync if h % 2 == 0 else nc.gpsimd
                eng.dma_start(diag_t[0][:, sl], d_diags.ap()[0][:, sl])
            nc.gpsimd.dma_start(gc_all[:, 384:768], d_gc.ap()[:, 384:768])
            nc.sync.dma_start(diag_t[1][:, 0:2048], d_diags.ap()[1][:, 0:2048])
            nc.gpsimd.dma_start(diag_t[1][:, 2048:4096], d_diags.ap()[1][:, 2048:4096])
            nc.sync.dma_start(gab_all[:, 1024:1536], d_gab.ap()[:, 1024:1536])
            nc.gpsimd.dma_start(gc_all[:, 768:1152], d_gc.ap()[:, 768:1152])
            nc.sync.dma_start(diag_t[2][:, 0:2048], d_diags.ap()[2][:, 0:2048])
            nc.gpsimd.dma_start(diag_t[2][:, 2048:4096], d_diags.ap()[2][:, 2048:4096])
            nc.sync.dma_start(gab_all[:, 1536:2048], d_gab.ap()[:, 1536:2048])
            nc.gpsimd.dma_start(gc_all[:, 1152:1536], d_gc.ap()[:, 1152:1536])
            nc.sync.dma_start(hp_t[:], d_hp.ap())

            def joined_view(tile_, g):
                # [P, j(2), c(re/im 2), h(128)] view of dst cols
                # c*2048 + 256*g + 128*j + h
                v = tile_[:].rearrange(
                    "p (c g j h) -> p g j c h", c=2, g=8, j=2
                )
                return v[:, g]

            def copy_group(engine, dst_tile, g, ps):
                src = ps[:].rearrange("p (j c h) -> p j c h", j=2, c=2)
                dst = joined_view(dst_tile, g)
                if engine == "v":
                    nc.vector.tensor_copy(dst, src)
                else:
                    nc.scalar.copy(dst, src)

            def a_group(t, g, src_tile, dst_tile):
                """Phase A/B matmul group g (2 windows) -> ps tile."""
                ps = ps_mm.tile([P, 512], FP32, tag="ps_mm")
                cd7 = gab_all[:, 512 * t : 512 * t + 256]
                ndc7 = gab_all[:, 512 * t + 256 : 512 * t + 512]
                for j in range(2):
                    w = 2 * g + j
                    out_sl = ps[:, 256 * j : 256 * (j + 1)]
                    re_w = src_tile[:, 128 * w : 128 * (w + 1)]
                    im_w = src_tile[:, 2048 + 128 * w : 2048 + 128 * (w + 1)]
                    nc.tensor.matmul(out_sl, re_w, cd7, start=True, stop=False)
                    nc.tensor.matmul(out_sl, im_w, ndc7, start=False, stop=True)
                return ps

            def b_group(t, g, src_tile):
                ps = ps_mm.tile([P, 512], FP32, tag="ps_mm")
                cd7 = gab_all[:, 512 * t : 512 * t + 256]
                ndc7 = gab_all[:, 512 * t + 256 : 512 * t + 512]
                sv = src_tile[:].rearrange("p (c x u) -> p c x u", c=2, x=128)
                for j in range(2):
                    w = 2 * g + j
                    out_sl = ps[:, 256 * j : 256 * (j + 1)]
                    nc.tensor.matmul(out_sl, sv[:, 0, :, w], cd7, start=True, stop=False)
                    nc.tensor.matmul(out_sl, sv[:, 1, :, w], ndc7, start=False, stop=True)
                return ps

            def c_chunk(t, k, src_tile):
                """Phase C chunk k: pc = [pre(512) | pim(512)]."""
                pc = ps_c.tile([P, 1024], FP32, tag="ps_c", name=f"pc{t}{k}")
                c41 = gc_all[:, 384 * t : 384 * t + 128]
                nd41 = gc_all[:, 384 * t + 128 : 384 * t + 256]
                d41 = gc_all[:, 384 * t + 256 : 384 * t + 384]
                ck_re = src_tile[:, 512 * k : 512 * (k + 1)]
                ck_im = src_tile[:, 2048 + 512 * k : 2048 + 512 * (k + 1)]
                pre = pc[:, 0:512]
                pim = pc[:, 512:1024]
                # LDW-minimizing order: c41 used by both accumulation groups
                nc.tensor.matmul(pre, c41, ck_re, start=True, stop=False)
                nc.tensor.matmul(pim, c41, ck_im, start=True, stop=False)
                nc.tensor.matmul(pre, nd41, ck_im, start=False, stop=True)
                nc.tensor.matmul(pim, d41, ck_re, start=False, stop=True)
                return pc

            def rot_chunk(t, k, pc, dst_tile):
                """Apply exp(-i hp) to C output chunk k -> dst state chunk k."""
                dia = diag_t[t - 1][:, 1024 * k : 1024 * (k + 1)]  # [cos|sin]
                cos_d = dia[:, 0:512]
                sin_d = dia[:, 512:1024]
                sc = r_pool.tile([P, 1024], FP16, tag="sc")
                nc.scalar.copy(sc[:], pc[:])
                s_re = sc[:, 0:512]
                s_im = sc[:, 512:1024]
                m1 = r_pool.tile([P, 1024], FP16, tag="m1")
                m2 = r_pool.tile([P, 1024], FP16, tag="m2")
                # m1 = [re*c | im*s] in one wide 2x op; re' = lo + hi
                nc.vector.tensor_tensor(m1[:], sc[:], dia, ALU.mult)
                nc.vector.tensor_tensor(m2[:, 0:512], s_im, cos_d, ALU.mult)
                nc.vector.tensor_tensor(m2[:, 512:1024], s_re, sin_d, ALU.mult)
                dst_re = dst_tile[:, 512 * k : 512 * (k + 1)]
                dst_im = dst_tile[:, 2048 + 512 * k : 2048 + 512 * (k + 1)]
                nc.vector.tensor_tensor(dst_re, m1[:, 0:512], m1[:, 512:1024], ALU.add)
                nc.vector.tensor_tensor(
                    dst_im, m2[:, 0:512], m2[:, 512:1024], ALU.subtract
                )

            # copy-engine assignment: "v" = vector, "s" = scalar
            # layers with rotation load DVE heavily -> scalar takes more copies
            A_ENG = {
                0: ["v", "s", "v", "s", "v", "s", "v", "s"],
                1: ["s", "v", "s", "s", "v", "s", "s", "v"],
            }
            B_ENG = {
                0: ["v", "s", "v", "s", "v", "s", "v", "s"],
                1: ["v", "s", "s", "v", "v", "s", "s", "v"],
            }

            hp_d = hp_t

            def energy_chunk(k, pc):
                ck = slice(512 * k, 512 * (k + 1))
                # one wide square (ScalarE) per chunk, then probs + weighted
                # accumulate on DVE; per-chunk tiles so chunks pipeline
                sq = r_pool.tile([P, 1024], FP16, tag="sq")
                pr = r_pool.tile([P, 512], FP16, tag="pr")
                nc.scalar.activation(sq[:], pc[:], ACT.Square)
                nc.vector.tensor_tensor(pr[:], sq[:, 0:512], sq[:, 512:1024], ALU.add)
                nc.vector.scalar_tensor_tensor(
                    sq[:, 0:512],  # dummy out, reused
                    pr[:],
                    1.0,
                    hp_d[:, ck],
                    ALU.mult,
                    ALU.mult,
                    accum_out=part_k[k][:],
                )

            for t in range(LAYERS):
                a_eng = A_ENG[min(t, 1)]
                b_eng = B_ENG[min(t, 1)]
                # ---- phase A (st_a ready: t=0 from DMA, else from the
                # rotation emitted inside layer t-1's B/C loop)
                for g in range(8):
                    ps = a_group(t, g, st_a, st_b)
                    copy_group(a_eng[g], st_b, g, ps)
                # hold HAM warm while the last A copies land (A->B barrier)
                dummy_mms(3)
                # ---- phase B interleaved with phase C + rotation: C chunk k
                # needs only B groups 2k,2k+1, but is emitted one B group
                # later so its B-copy dependency has already landed; this
                # spreads the rotation chain across the whole B+C stretch.
                seq = ["b0", "b1", "b2", "c0", "b3", "b4", "c1",
                       "b5", "b6", "c2", "b7", "c3"]
                for item in seq:
                    if item[0] == "b":
                        g = int(item[1])
                        ps = b_group(t, g, st_b)
                        copy_group(b_eng[g], st_a, g, ps)
                    else:
                        k = int(item[1])
                        if k == 0:
                            # hold HAM warm while B0/B1 copies land
                            dummy_mms(3)
                        pc = c_chunk(t, k, st_a)
                        if t < LAYERS - 1:
                            rot_chunk(t + 1, k, pc, st_a)
                        else:
                            energy_chunk(k, pc)
                # hold HAM warm while the first rotation chunk finishes
                if t < LAYERS - 1:
                    dummy_mms(6)
            nc.vector.tensor_tensor(comb[0][:], part_k[0][:], part_k[1][:], ALU.add)
            nc.vector.tensor_tensor(comb[1][:], part_k[2][:], part_k[3][:], ALU.add)
            nc.vector.tensor_tensor(comb[2][:], comb[0][:], comb[1][:], ALU.add)
            nc.sync.dma_start(d_out.ap(), comb[2][:])

    nc.compile()
    return nc


_NC_CACHE = {}


def _get_program():
    if "nc" not in _NC_CACHE:
        _NC_CACHE["nc"] = _build_program()
    return _NC_CACHE["nc"]


def kernel(batch_betas, adj_matrices, _trace=False, _tmpdir=None):
    batch_betas = np.asarray(batch_betas, dtype=np.float32)
    adj_matrices = np.asarray(adj_matrices, dtype=np.float32)
    assert batch_betas.shape == (BATCH, LAYERS)
    assert adj_matrices.shape == (BATCH, N, N)

    nc = _get_program()
    in_maps = _host_prep(batch_betas, adj_matrices)
    res = run_bass_kernel_spmd(
        nc,
        in_maps,
        list(range(NCORES)),
        trace=_trace,
        tmpdir=_tmpdir,
    )
    energies = np.array(
        [res.results[b]["out"].sum() / DIM for b in range(BATCH)], dtype=np.float32
    )
    if _trace:
        return energies, res
    return energies


# revision 60
# speedup vs baseline: 1.0108x; 1.0108x over previous
"""TRN2 Bass kernel for nn_DiffQuantumSimulator (QAOA MaxCut, 18 qubits, p=4).

Strategy: data-parallel over batch (8 graphs -> 8 NeuronCores). Per core the
2^18 statevector lives in SBUF as one [128 x 4096] fp16 tile (re | im halves).

Each QAOA layer applies exp(-i*hp) (diagonal, elementwise) and the mixer
RX(beta)^(x)18 in 3 TensorE matmul phases:
  A: 128x128 complex gate RX^(x)7 on the 7 partition bits, fused with a
     partition<->free-bit transpose by using the *state* as the stationary
     operand (out = state_tile^T @ [C|D]).
  B: same trick on the next 7 bits (stride-16 windows).
  C: standard matmul applying RX^(x)4 (x) I_8 to the remaining 4 bits.
All matmuls run in fp16 (1 col/cycle at 2.4 GHz warm). The PE is pre-warmed
with dummy matmuls during the input-DMA head, and small fillers at the two
per-layer stall points hold the HAM clock-gate at K=8/8.

The diagonal rotation (layers 2..4) reads phase C's PSUM output: ScalarE
stages PSUM->SBUF fp16 (one [P,1024] copy per chunk), then VectorE applies
exp(-i hp) with one wide + two half multiplies and two add/subs, all in
16-bit 2x mode. C chunks and their rotations are emitted interleaved with
the B groups so the chain overlaps the whole B+C matmul stretch. PSUM->SBUF
copies for phases A/B are split across ScalarE/VectorE; input DMAs are
issued only from sync (HWDGE) and gpsimd (SWDGE) so the compute queues
never block on transfers.

Diagonals (cos/sin of hp per layer, in the layer's bit-layout), the gate
matrices, and hp itself are precomputed on host from the runtime inputs.
Device returns per-partition energy partial sums; host reduces and scales.
"""

import numpy as np

import concourse.bass as bass
import concourse.mybir as mybir
import concourse.tile as tile
from concourse import bacc
from concourse.bass_utils import run_bass_kernel_spmd

N = 18
DIM = 1 << N
P = 128
F = DIM // P  # 2048
LAYERS = 4
BATCH = 8
NCORES = 8

FP32 = mybir.dt.float32
FP16 = mybir.dt.float16
ALU = mybir.AluOpType
ACT = mybir.ActivationFunctionType

# ----------------------------------------------------------------------------
# Host-side math: hp diagonal, gate matrices, bit-layout permutations
# ----------------------------------------------------------------------------


def _compute_hp(adj):
    W = (np.triu(adj, k=1) > 0.5).astype(np.float64)
    n_edges = W.sum()
    idx = np.arange(DIM)
    shifts = (N - 1 - np.arange(N))[:, None]
    Z = 1.0 - 2.0 * ((idx[None, :] >> shifts) & 1).astype(np.float64)
    T = W @ Z
    cross = np.einsum("ud,ud->d", T, Z)
    return 0.5 * (n_edges - cross)  # [DIM], integer-valued*0.5, exact


def _rx(beta):
    c, s = np.cos(beta), np.sin(beta)
    return np.array([[c, -1j * s], [-1j * s, c]], dtype=np.complex128)


def _kron_list(mats):
    out = np.array([[1.0]], dtype=np.complex128)
    for m in mats:
        out = np.kron(out, m)
    return out


def _m7(beta):
    return _kron_list([_rx(beta)] * 7)


def _m41(beta):
    return _kron_list([_rx(beta)] * 4 + [np.eye(2, dtype=np.complex128)] * 3)


def _bitmap_after_A(bm):
    new = [0] * N
    for j in range(7):
        new[11 + j] = bm[j]
    for j in range(4):
        new[7 + j] = bm[7 + j]
    for j in range(7):
        new[j] = bm[11 + j]
    return new


def _bitmap_after_B(bm):
    # window = free bits 10..4 (stride-16 single AP dim), tiles = bits 3..0
    new = [0] * N
    for j in range(7):
        new[11 + j] = bm[4 + j]
    for j in range(4):
        new[7 + j] = bm[j]
    for j in range(7):
        new[j] = bm[11 + j]
    return new


def _perm_for_bitmap(bm):
    a = np.arange(DIM, dtype=np.int64)
    out = np.zeros(DIM, dtype=np.int64)
    for j in range(N):
        out |= ((a >> j) & 1) << bm[j]
    return out


def _layer_perms():
    """Permutations (orig_idx = perm[cur_idx]) for the state layout at the
    start of each layer (1..LAYERS) plus the final layout (index LAYERS)."""
    perms = []
    bm = list(range(N))
    for _ in range(LAYERS):
        perms.append(_perm_for_bitmap(bm))
        bm = _bitmap_after_B(_bitmap_after_A(bm))
    perms.append(_perm_for_bitmap(bm))
    return perms


_PERMS = _layer_perms()


def _host_prep(batch_betas, adj_matrices):
    """Build per-core input dicts."""
    in_maps = []
    for b in range(BATCH):
        hp = _compute_hp(np.asarray(adj_matrices[b], dtype=np.float64))
        cos_hp = np.cos(hp)
        sin_hp = np.sin(hp)

        # init state (= exp(-i hp) applied to unnormalized uniform state),
        # packed [P, 4096] = [re(2048) | im(2048)], matching the st_a layout
        init = np.empty((P, 4096), dtype=np.float16)
        init[:, 0:2048] = cos_hp[_PERMS[0]].reshape(P, F)
        init[:, 2048:4096] = (-sin_hp[_PERMS[0]]).reshape(P, F)

        # diag planes per rotation layer: dA = [cos|sin] per 512-chunk
        # (matches the C-output [re|im] chunk layout for one-op multiplies)
        diags = np.empty((LAYERS - 1, P, 4 * 1024), dtype=np.float16)
        for t in range(1, LAYERS):
            c = cos_hp[_PERMS[t]].reshape(P, 4, 512)
            s = sin_hp[_PERMS[t]].reshape(P, 4, 512)
            dA = diags[t - 1].reshape(P, 4, 2, 512)
            dA[:, :, 0, :] = c
            dA[:, :, 1, :] = s
        hp_plane = hp[_PERMS[LAYERS]].reshape(P, F).astype(np.float16)

        gates_ab = np.empty((P, LAYERS * 512), dtype=np.float16)
        gates_c = np.empty((P, LAYERS * 384), dtype=np.float16)
        for t in range(LAYERS):
            beta = float(np.asarray(batch_betas[b][t], dtype=np.float64))
            M7 = _m7(beta)
            C7 = M7.real
            D7 = M7.imag
            M41 = _m41(beta)
            C41 = M41.real
            D41 = M41.imag
            o = 512 * t
            gates_ab[:, o + 0 : o + 128] = C7
            gates_ab[:, o + 128 : o + 256] = D7
            gates_ab[:, o + 256 : o + 384] = -D7
            gates_ab[:, o + 384 : o + 512] = C7
            o = 384 * t
            gates_c[:, o + 0 : o + 128] = C41
            gates_c[:, o + 128 : o + 256] = -D41
            gates_c[:, o + 256 : o + 384] = D41

        in_maps.append(
            {
                "init": init,
                "diags": diags,
                "hp": hp_plane,
                "gates_ab": gates_ab,
                "gates_c": gates_c,
            }
        )
    return in_maps


# ----------------------------------------------------------------------------
# Bass program
# ----------------------------------------------------------------------------

N_WARMUP_MM = 6  # dummy matmuls (N=512) to warm HAM during the DMA head
ROT_ADD_GPSIMD = False  # GpSimd TT contends with DVE SBUF ports: net loss


def _build_program():
    nc = bacc.Bacc("TRN2", target_bir_lowering=False, debug=False)

    d_init = nc.dram_tensor("init", [P, 4096], FP16, kind="ExternalInput")
    d_diags = nc.dram_tensor(
        "diags", [LAYERS - 1, P, 4096], FP16, kind="ExternalInput"
    )
    d_hp = nc.dram_tensor("hp", [P, F], FP16, kind="ExternalInput")
    d_gab = nc.dram_tensor("gates_ab", [P, LAYERS * 512], FP16, kind="ExternalInput")
    d_gc = nc.dram_tensor("gates_c", [P, LAYERS * 384], FP16, kind="ExternalInput")
    d_out = nc.dram_tensor("out", [P, 1], FP32, kind="ExternalOutput")

    with tile.TileContext(nc) as tc:
        with (
            tc.tile_pool(name="state", bufs=1) as st_pool,
            tc.tile_pool(name="consts", bufs=1) as c_pool,
            tc.tile_pool(name="rot", bufs=2) as r_pool,
            tc.tile_pool(name="scratch", bufs=1) as s_pool,
            tc.tile_pool(name="ps_mm", bufs=3, space="PSUM") as ps_mm,
            tc.tile_pool(name="ps_c", bufs=2, space="PSUM") as ps_c,
            tc.tile_pool(name="ps_d", bufs=1, space="PSUM") as ps_d,
        ):
            # state tiles: [re(2048) | im(2048)]
            st_a = st_pool.tile([P, 2 * F], FP16, tag="st_a")
            st_b = st_pool.tile([P, 2 * F], FP16, tag="st_b")

            diag_t = [
                c_pool.tile([P, 4096], FP16, tag=f"diag{t}", name=f"diag{t}")
                for t in range(LAYERS - 1)
            ]
            hp_t = c_pool.tile([P, F], FP16, tag="hp")
            gab_all = c_pool.tile([P, LAYERS * 512], FP16, tag="gab")
            gc_all = c_pool.tile([P, LAYERS * 384], FP16, tag="gc")

            # rotation scratch (per chunk, double-buffered via pool bufs)
            wu = s_pool.tile([P, 512], FP16, tag="wu")
            sq_re = s_pool.tile([P, 512], FP16, tag="sq_re")
            sq_im = s_pool.tile([P, 512], FP16, tag="sq_im")
            probs = s_pool.tile([P, 512], FP16, tag="probs")
            part_k = [
                s_pool.tile([P, 1], FP32, tag=f"part{k}", name=f"part{k}")
                for k in range(4)
            ]
            comb = [
                s_pool.tile([P, 1], FP32, tag=f"comb{k}", name=f"comb{k}")
                for k in range(3)
            ]

            # ---- PE warmup / HAM-hold fillers: dummy matmuls on zeroed SBUF
            # into a dedicated PSUM bank (no deps on real work)
            nc.vector.memset(wu[:], 0.0)
            dps = ps_d.tile([P, 512], FP32, tag="ps_d")

            def dummy_mms(n, cols=256):
                for _ in range(n):
                    nc.tensor.matmul(
                        dps[:, 0:cols], wu[:, 0:128], wu[:, 0:cols],
                        start=True, stop=True,
                    )

            dummy_mms(N_WARMUP_MM, cols=512)

            # ---- input DMAs. NONE on scalar/vector (their queues must stay
            # free for compute). sync = HWDGE, gpsimd = SWDGE. Chunked so the
            # first pieces arrive early and phase A can start streaming.
            nc.sync.dma_start(gab_all[:, 0:512], d_gab.ap()[:, 0:512])
            nc.gpsimd.dma_start(st_a[:, 2048:2560], d_init.ap()[:, 2048:2560])
            for k in range(4):
                nc.sync.dma_start(
                    st_a[:, 512 * k : 512 * (k + 1)],
                    d_init.ap()[:, 512 * k : 512 * (k + 1)],
                )
                if k > 0:
                    nc.gpsimd.dma_start(
                        st_a[:, 2048 + 512 * k : 2048 + 512 * (k + 1)],
                        d_init.ap()[:, 2048 + 512 * k : 2048 + 512 * (k + 1)],
                    )
            nc.sync.dma_start(gc_all[:, 0:384], d_gc.ap()[:, 0:384])
            nc.sync.dma_start(gab_all[:, 512:1024], d_gab.ap()[:, 512:1024])
            # layer-2 rotation diag, chunked for earliest first-chunk arrival
            for h in range(4):
                sl = slice(1024 * h, 1024 * (h + 1))
                eng = nc.sync if h % 2 == 0 else nc.gpsimd
                eng.dma_start(diag_t[0][:, sl], d_diags.ap()[0][:, sl])
            nc.gpsimd.dma_start(gc_all[:, 384:768], d_gc.ap()[:, 384:768])
            nc.sync.dma_start(diag_t[1][:, 0:2048], d_diags.ap()[1][:, 0:2048])
            nc.gpsimd.dma_start(diag_t[1][:, 2048:4096], d_diags.ap()[1][:, 2048:4096])
            nc.sync.dma_start(gab_all[:, 1024:1536], d_gab.ap()[:, 1024:1536])
            nc.gpsimd.dma_start(gc_all[:, 768:1152], d_gc.ap()[:, 768:1152])
            nc.sync.dma_start(diag_t[2][:, 0:2048], d_diags.ap()[2][:, 0:2048])
            nc.gpsimd.dma_start(diag_t[2][:, 2048:4096], d_diags.ap()[2][:, 2048:4096])
            nc.sync.dma_start(gab_all[:, 1536:2048], d_gab.ap()[:, 1536:2048])
            nc.gpsimd.dma_start(gc_all[:, 1152:1536], d_gc.ap()[:, 1152:1536])
            nc.sync.dma_start(hp_t[:], d_hp.ap())

            def joined_view(tile_, g):
                # [P, j(2), c(re/im 2), h(128)] view of dst cols
                # c*2048 + 256*g + 128*j + h
                v = tile_[:].rearrange(
                    "p (c g j h) -> p g j c h", c=2, g=8, j=2
                )
                return v[:, g]

            def copy_group(engine, dst_tile, g, ps):
                src = ps[:].rearrange("p (j c h) -> p j c h", j=2, c=2)
                dst = joined_view(dst_tile, g)
                if engine == "v":
                    nc.vector.tensor_copy(dst, src)
                else:
                    nc.scalar.copy(dst, src)

            def a_group(t, g, src_tile, dst_tile):
                """Phase A/B matmul group g (2 windows) -> ps tile."""
                ps = ps_mm.tile([P, 512], FP32, tag="ps_mm")
                cd7 = gab_all[:, 512 * t : 512 * t + 256]
                ndc7 = gab_all[:, 512 * t + 256 : 512 * t + 512]
                for j in range(2):
                    w = 2 * g + j
                    out_sl = ps[:, 256 * j : 256 * (j + 1)]
                    re_w = src_tile[:, 128 * w : 128 * (w + 1)]
                    im_w = src_tile[:, 2048 + 128 * w : 2048 + 128 * (w + 1)]
                    nc.tensor.matmul(out_sl, re_w, cd7, start=True, stop=False)
                    nc.tensor.matmul(out_sl, im_w, ndc7, start=False, stop=True)
                return ps

            def b_group(t, g, src_tile):
                ps = ps_mm.tile([P, 512], FP32, tag="ps_mm")
                cd7 = gab_all[:, 512 * t : 512 * t + 256]
                ndc7 = gab_all[:, 512 * t + 256 : 512 * t + 512]
                sv = src_tile[:].rearrange("p (c x u) -> p c x u", c=2, x=128)
                for j in range(2):
                    w = 2 * g + j
                    out_sl = ps[:, 256 * j : 256 * (j + 1)]
                    nc.tensor.matmul(out_sl, sv[:, 0, :, w], cd7, start=True, stop=False)
                    nc.tensor.matmul(out_sl, sv[:, 1, :, w], ndc7, start=False, stop=True)
                return ps

            def c_chunk(t, k, src_tile):
                """Phase C chunk k: pc = [pre(512) | pim(512)]."""
                pc = ps_c.tile([P, 1024], FP32, tag="ps_c", name=f"pc{t}{k}")
                c41 = gc_all[:, 384 * t : 384 * t + 128]
                nd41 = gc_all[:, 384 * t + 128 : 384 * t + 256]
                d41 = gc_all[:, 384 * t + 256 : 384 * t + 384]
                ck_re = src_tile[:, 512 * k : 512 * (k + 1)]
                ck_im = src_tile[:, 2048 + 512 * k : 2048 + 512 * (k + 1)]
                pre = pc[:, 0:512]
                pim = pc[:, 512:1024]
                # LDW-minimizing order: c41 used by both accumulation groups
                nc.tensor.matmul(pre, c41, ck_re, start=True, stop=False)
                nc.tensor.matmul(pim, c41, ck_im, start=True, stop=False)
                nc.tensor.matmul(pre, nd41, ck_im, start=False, stop=True)
                nc.tensor.matmul(pim, d41, ck_re, start=False, stop=True)
                return pc

            def rot_chunk(t, k, pc, dst_tile):
                """Apply exp(-i hp) to C output chunk k -> dst state chunk k."""
                dia = diag_t[t - 1][:, 1024 * k : 1024 * (k + 1)]  # [cos|sin]
                cos_d = dia[:, 0:512]
                sin_d = dia[:, 512:1024]
                sc = r_pool.tile([P, 1024], FP16, tag="sc")
                nc.scalar.copy(sc[:], pc[:])
                s_re = sc[:, 0:512]
                s_im = sc[:, 512:1024]
                m1 = r_pool.tile([P, 1024], FP16, tag="m1")
                m2 = r_pool.tile([P, 1024], FP16, tag="m2")
                # m1 = [re*c | im*s] in one wide 2x op; re' = lo + hi
                nc.vector.tensor_tensor(m1[:], sc[:], dia, ALU.mult)
                nc.vector.tensor_tensor(m2[:, 0:512], s_im, cos_d, ALU.mult)
                nc.vector.tensor_tensor(m2[:, 512:1024], s_re, sin_d, ALU.mult)
                dst_re = dst_tile[:, 512 * k : 512 * (k + 1)]
                dst_im = dst_tile[:, 2048 + 512 * k : 2048 + 512 * (k + 1)]
                nc.vector.tensor_tensor(dst_re, m1[:, 0:512], m1[:, 512:1024], ALU.add)
                nc.vector.tensor_tensor(
                    dst_im, m2[:, 0:512], m2[:, 512:1024], ALU.subtract
                )

            # copy-engine assignment: "v" = vector, "s" = scalar
            # layers with rotation load DVE heavily -> scalar takes more copies
            A_ENG = {
                0: ["v", "s", "v", "s", "v", "s", "v", "s"],
                1: ["s", "v", "s", "s", "v", "s", "s", "v"],
            }
            B_ENG = {
                0: ["v", "s", "v", "s", "v", "s", "v", "s"],
                1: ["v", "s", "s", "s", "v", "s", "s", "v"],
            }

            hp_d = hp_t

            def energy_chunk(k, pc):
                ck = slice(512 * k, 512 * (k + 1))
                # one wide square (ScalarE) per chunk, then probs + weighted
                # accumulate on DVE; per-chunk tiles so chunks pipeline
                sq = r_pool.tile([P, 1024], FP16, tag="sq")
                pr = r_pool.tile([P, 512], FP16, tag="pr")
                nc.scalar.activation(sq[:], pc[:], ACT.Square)
                nc.vector.tensor_tensor(pr[:], sq[:, 0:512], sq[:, 512:1024], ALU.add)
                nc.vector.scalar_tensor_tensor(
                    sq[:, 0:512],  # dummy out, reused
                    pr[:],
                    1.0,
                    hp_d[:, ck],
                    ALU.mult,
                    ALU.mult,
                    accum_out=part_k[k][:],
                )

            for t in range(LAYERS):
                a_eng = A_ENG[min(t, 1)]
                b_eng = B_ENG[min(t, 1)]
                # ---- phase A (st_a ready: t=0 from DMA, else from the
                # rotation emitted inside layer t-1's B/C loop)
                for g in range(8):
                    ps = a_group(t, g, st_a, st_b)
                    copy_group(a_eng[g], st_b, g, ps)
                # hold HAM warm while the last A copies land (A->B barrier)
                dummy_mms(3)
                # ---- phase B interleaved with phase C + rotation: C chunk k
                # needs only B groups 2k,2k+1, but is emitted one B group
                # later so its B-copy dependency has already landed; this
                # spreads the rotation chain across the whole B+C stretch.
                seq = ["b0", "b1", "b2", "c0", "b3", "b4", "c1",
                       "b5", "b6", "c2", "b7", "c3"]
                for item in seq:
                    if item[0] == "b":
                        g = int(item[1])
                        ps = b_group(t, g, st_b)
                        copy_group(b_eng[g], st_a, g, ps)
                    else:
                        k = int(item[1])
                        if k == 0:
                            # hold HAM warm while B0/B1 copies land
                            dummy_mms(3)
                        pc = c_chunk(t, k, st_a)
                        if t < LAYERS - 1:
                            rot_chunk(t + 1, k, pc, st_a)
                        else:
                            energy_chunk(k, pc)
                # hold HAM warm while the first rotation chunk finishes
                if t < LAYERS - 1:
                    dummy_mms(6)
            nc.vector.tensor_tensor(comb[0][:], part_k[0][:], part_k[1][:], ALU.add)
            nc.vector.tensor_tensor(comb[1][:], part_k[2][:], part_k[3][:], ALU.add)
            nc.vector.tensor_tensor(comb[2][:], comb[0][:], comb[1][:], ALU.add)
            nc.sync.dma_start(d_out.ap(), comb[2][:])

    nc.compile()
    return nc


_NC_CACHE = {}


def _get_program():
    if "nc" not in _NC_CACHE:
        _NC_CACHE["nc"] = _build_program()
    return _NC_CACHE["nc"]


def kernel(batch_betas, adj_matrices, _trace=False, _tmpdir=None):
    batch_betas = np.asarray(batch_betas, dtype=np.float32)
    adj_matrices = np.asarray(adj_matrices, dtype=np.float32)
    assert batch_betas.shape == (BATCH, LAYERS)
    assert adj_matrices.shape == (BATCH, N, N)

    nc = _get_program()
    in_maps = _host_prep(batch_betas, adj_matrices)
    res = run_bass_kernel_spmd(
        nc,
        in_maps,
        list(range(NCORES)),
        trace=_trace,
        tmpdir=_tmpdir,
    )
    energies = np.array(
        [res.results[b]["out"].sum() / DIM for b in range(BATCH)], dtype=np.float32
    )
    if _trace:
        return energies, res
    return energies
